# revision 37
# baseline (speedup 1.0000x reference)
"""DGAT (dynamic-weight GAT) Trainium2 kernel, 8-core edge/node-parallel.

Strategy:
  - Host: sort edges by destination node; shard nodes (and their incoming
    edges) across 8 cores in fixed 2500-node ranges -> fully disjoint, no
    collectives. Within a core, nodes are processed in 128-node windows;
    each window's edges are padded to G*128 slots (G = compile-time max).
  - Device per 128-edge subtile:
      layer1 (PE)  : h_src/h_dst = relu(ef @ w1 + b1)   [64, 512] psum
      W+V (PE)     : [w2d_om | A1d | A1s] matmuls with ones-row bias trick
      bmm (DVE)    : per-edge x_j via broadcast-mult, att-contractions for
                     raw logits via host-precomputed A1 matrices
      softmax      : exp without segment-max (raw range is bounded ~[-3,11]),
                     denominator accumulated alongside messages
      aggregation  : one-hot S matrix (iota==dst_rel) segment-matmul on PE,
                     accumulating G subtiles into a [128 nodes, 1028] PSUM
                     tile; per-window reduce + self-loop terms + divide.
"""
import sys
import os

sys.path.insert(0, "/opt/trn_rl_repo")

import numpy as np
from contextlib import ExitStack

import concourse.bass as bass
import concourse.tile as tile
import concourse.mybir as mybir
from concourse import bacc
from concourse.bass_utils import run_bass_kernel_spmd

F32 = mybir.dt.float32
F32R = mybir.dt.float32r
I16 = mybir.dt.int16
AF = mybir.ActivationFunctionType
OP = mybir.AluOpType
X = mybir.AxisListType.X

N, IN, E = 20000, 32, 200000
EDGE_IN, HID, HEADS, OUT = 16, 64, 4, 8
HO = HEADS * OUT
NEG = 0.2
NCORES = 8


def _host_prep(inp, npc, nwin, ncores):
    """Sort/shard/pad edges; build all per-core device arrays."""
    x = np.ascontiguousarray(inp["x"], dtype=np.float32)
    ef = np.ascontiguousarray(inp["edge_feats"], dtype=np.float32)
    ei = np.asarray(inp["edge_index"]).astype(np.int64)
    dst, src = ei[0], ei[1]
    nn = x.shape[0]
    ne = ef.shape[0]

    order = np.argsort(dst, kind="stable")
    dst_s = dst[order]
    src_s = src[order]
    core_of = dst_s // npc
    win_of = (dst_s % npc) // 128
    gwin = core_of * nwin + win_of
    # index within each (core, window) group (edges are sorted by gwin)
    counts = np.bincount(gwin, minlength=ncores * nwin)
    starts = np.r_[0, np.cumsum(counts)][:-1]
    idx_in_win = np.arange(ne) - starts[gwin]
    G = int(np.ceil(counts.max() / 128))
    nsub = nwin * G
    ntile = (nsub + 3) // 4
    nsub = ntile * 4  # pad subtiles to gather-tile granularity
    slots = ntile * 512

    slot_src = np.zeros((ncores, slots), np.int16)
    slot_rel = np.full((ncores, slots), -1.0, np.float32)
    slot_ef = np.zeros((ncores, slots, EDGE_IN), np.float32)
    s_global = win_of * (G * 128) + idx_in_win
    slot_src[core_of, s_global] = src_s.astype(np.int16)
    slot_rel[core_of, s_global] = ((dst_s % npc) % 128).astype(np.float32)
    slot_ef[core_of, s_global] = ef[order]

    # transposed layout: feature on partitions 0-15, slot on free dim
    ef_p = np.ascontiguousarray(slot_ef.transpose(0, 2, 1))  # [ncores, 16, slots]
    gidx16 = slot_src.reshape(ncores, slots // 16, 16).transpose(0, 2, 1)
    gidx = np.ascontiguousarray(
        np.tile(gidx16, (1, 8, 1))
    )  # [ncores, 128, slots/16] — idx pattern replicated per Q7 core
    drel = np.ascontiguousarray(
        slot_rel.reshape(ncores, nsub, 128).transpose(0, 2, 1)
    )  # [ncores, 128, nsub]

    # x padded to 256B rows for dma_gather
    xpad = np.zeros((nn, 64), np.float32)
    xpad[:, :IN] = x

    # per-core x windows, pre-arranged [p, (w, f)]
    node_slots = nwin * 128
    xwin = np.zeros((ncores, 128, nwin * IN), np.float32)
    for k in range(ncores):
        lo = k * npc
        take = min(npc, nn - lo)
        xw = np.zeros((node_slots, IN), np.float32)
        xw[:take] = x[lo : lo + take]
        xwin[k] = xw.reshape(nwin, 128, IN).transpose(1, 0, 2).reshape(128, nwin * IN)

    f32 = lambda k: np.asarray(inp[k], dtype=np.float32)
    w1s, w1d = f32("src_w1"), f32("dst_w1")
    b1s, b1d = f32("src_b1"), f32("dst_b1")
    w2s, w2d = f32("src_w2"), f32("dst_w2")
    b2s, b2d = f32("src_b2"), f32("dst_b2")
    selfw = f32("self_weights")
    att = f32("att")[0]  # [4, 16]
    bias = f32("bias")
    att1, att2 = att[:, :OUT], att[:, OUT:]

    w1cat = np.concatenate([w1s, w1d], axis=1).astype(np.float32)  # [16, 128]
    b1two = np.stack([b1s, b1d], axis=1).astype(np.float32)  # [64, 2]

    # o-major w2d + bias row
    w2d_om = w2d.reshape(HID, IN, HO).transpose(0, 2, 1).reshape(HID, HO * IN)
    b2d_om = b2d.reshape(IN, HO).T.reshape(1, HO * IN)
    # A1 matrices (h-major, i-inner) + bias rows
    w2s4 = w2s.reshape(HID, IN, HEADS, OUT)
    w2d4 = w2d.reshape(HID, IN, HEADS, OUT)
    A1d = np.einsum("zihg,hg->zhi", w2d4, att2).reshape(HID, HEADS * IN)
    A1s = np.einsum("zihg,hg->zhi", w2s4, att1).reshape(HID, HEADS * IN)
    A1d_b = np.einsum("ihg,hg->hi", b2d.reshape(IN, HEADS, OUT), att2).reshape(1, -1)
    A1s_b = np.einsum("ihg,hg->hi", b2s.reshape(IN, HEADS, OUT), att1).reshape(1, -1)
    wvW = np.concatenate(
        [
            np.concatenate([w2d_om, b2d_om], 0),
            np.concatenate([A1d, A1d_b], 0),
            np.concatenate([A1s, A1s_b], 0),
        ],
        axis=1,
    ).astype(np.float32)  # [65, 1280]

    rawsW = np.einsum(
        "ihg,hg->ih", selfw.reshape(IN, HEADS, OUT), att1 + att2
    ).astype(np.float32)
    selfcat = np.concatenate([selfw, rawsW], axis=1).astype(np.float32)  # [32, 36]

    ident = np.eye(128, dtype=np.float32)
    iota_r = np.tile(np.arange(128, dtype=np.float32)[None, :], (128, 1))
    brep = np.tile(bias[None, :], (128, 1)).astype(np.float32)

    in_maps = []
    for k in range(ncores):
        in_maps.append(
            {
                "ef_p": np.ascontiguousarray(ef_p[k]),
                "xpad": xpad,
                "gidx": np.ascontiguousarray(gidx[k]),
                "drel": np.ascontiguousarray(drel[k]),
                "xwin": np.ascontiguousarray(xwin[k]),
                "w1cat": w1cat,
                "b1two": b1two,
                "wvW": wvW,
                "selfcat": selfcat,
                "ident": ident,
                "iota_r": iota_r,
                "brep": brep,
                "ones_r": np.ones((1, 512), np.float32),
            }
        )
    meta = dict(G=G, nsub=nsub, ntile=ntile, nwin=nwin, npc=npc, nn=nn)
    return in_maps, meta


def _build_program(G, nsub, ntile, nwin, npc, nn):
    BF16 = bool(int(os.environ.get("KERNEL_BF16", "1")))
    BF = mybir.dt.float16
    WDT = BF if BF16 else F32
    ESHIFT = -5.0 if BF16 else 0.0
    nc = bacc.Bacc("TRN2", target_bir_lowering=False, debug=False)
    node_slots = nwin * 128

    ef_d = nc.dram_tensor("ef_p", [16, ntile * 512], F32R, kind="ExternalInput")
    xpad_d = nc.dram_tensor("xpad", [nn, 64], F32, kind="ExternalInput")
    gidx_d = nc.dram_tensor("gidx", [128, ntile * 32], I16, kind="ExternalInput")
    drel_d = nc.dram_tensor("drel", [128, nsub], F32, kind="ExternalInput")
    xwin_d = nc.dram_tensor("xwin", [128, nwin * IN], F32, kind="ExternalInput")
    w1cat_d = nc.dram_tensor("w1cat", [16, 128], F32R, kind="ExternalInput")
    b1two_d = nc.dram_tensor("b1two", [64, 2], F32, kind="ExternalInput")
    wvW_d = nc.dram_tensor("wvW", [65, 1280], F32R, kind="ExternalInput")
    selfcat_d = nc.dram_tensor("selfcat", [32, 36], F32, kind="ExternalInput")
    ident_d = nc.dram_tensor("ident", [128, 128], F32, kind="ExternalInput")
    iota_d = nc.dram_tensor("iota_r", [128, 128], F32, kind="ExternalInput")
    brep_d = nc.dram_tensor("brep", [128, 32], F32, kind="ExternalInput")
    ones_d = nc.dram_tensor("ones_r", [1, 512], F32R, kind="ExternalInput")
    out_d = nc.dram_tensor("out_c", [node_slots, IN], F32, kind="ExternalOutput")

    with tile.TileContext(nc) as tc, ExitStack() as ctx:
        const = ctx.enter_context(tc.tile_pool(name="const", bufs=1))
        xgp = ctx.enter_context(tc.tile_pool(name="xgp", bufs=2))
        hsb = ctx.enter_context(tc.tile_pool(name="hsb", bufs=2))
        wvs = ctx.enter_context(tc.tile_pool(name="wvs", bufs=2))
        sml = ctx.enter_context(tc.tile_pool(name="sml", bufs=3))
        t3p = ctx.enter_context(tc.tile_pool(name="t3p", bufs=2))
        outp = ctx.enter_context(tc.tile_pool(name="outp", bufs=2))
        ps_h = ctx.enter_context(tc.tile_pool(name="ps_h", bufs=1, space="PSUM"))
        ps_w = ctx.enter_context(tc.tile_pool(name="ps_w", bufs=2, space="PSUM"))
        ps_v = ctx.enter_context(tc.tile_pool(name="ps_v", bufs=1, space="PSUM"))
        ps_ag = ctx.enter_context(tc.tile_pool(name="ps_ag", bufs=1, space="PSUM"))
        ps_dn = ctx.enter_context(tc.tile_pool(name="ps_dn", bufs=1, space="PSUM"))

        def load_const(name, dram, shape, dtype=F32):
            t = const.tile(shape, dtype, tag=name)
            nc.sync.dma_start(t[:], dram[:])
            return t

        ef_sb = load_const("ef_sb", ef_d, [16, ntile * 512], F32R)
        gidx_sb = load_const("gidx_sb", gidx_d, [128, ntile * 32], I16)
        drel_sb = load_const("drel_sb", drel_d, [128, nsub])
        xwin_sb = load_const("xwin_sb", xwin_d, [128, nwin * IN])
        w1cat_sb = load_const("w1cat_sb", w1cat_d, [16, 128], F32R)
        b1two_sb = load_const("b1two_sb", b1two_d, [64, 2])
        wvW_sb = load_const("wvW_sb", wvW_d, [65, 1280], F32R)
        selfcat_sb = load_const("selfcat_sb", selfcat_d, [32, 36])
        ident_sb = load_const("ident_sb", ident_d, [128, 128])
        iota_sb = load_const("iota_sb", iota_d, [128, 128])
        brep_sb = load_const("brep_sb", brep_d, [128, 32])
        esh_sb = const.tile([128, 1], F32, tag="esh")
        nc.gpsimd.memset(esh_sb[:], ESHIFT)

        xg4 = None
        hs_sb = hd_sb = None
        aggr_ps = None

        sublimit = int(os.environ.get("KERNEL_SUBLIMIT", str(nsub)))
        oplimit = int(os.environ.get("KERNEL_OPLIMIT", "99"))
        for sub in range(min(nsub, sublimit)):
            t, c = sub // 4, sub % 4
            w, j = sub // G, sub % G
            if w >= nwin:
                break
            if c == 0:
                # gather 512 edges' x rows
                xg4 = xgp.tile([128, 4, 64], F32, tag="xg4")
                nc.gpsimd.dma_gather(
                    xg4[:], xpad_d[:], gidx_sb[:, t * 32 : (t + 1) * 32], 512, 512, 64
                )
                if oplimit < 2:
                    continue
                # layer 1: one N=512 f32r matmul per net -> h [64, 512] psum
                hs_ps = ps_h.tile([64, 512], F32, tag="hs")
                hd_ps = ps_h.tile([64, 512], F32, tag="hd")
                rhs = ef_sb[:, t * 512 : (t + 1) * 512]
                for net, ps in ((0, hs_ps), (1, hd_ps)):
                    nc.tensor.matmul(
                        ps[:],
                        w1cat_sb[:, net * 64 : (net + 1) * 64],
                        rhs,
                        start=True,
                        stop=True,
                    )
                if oplimit < 3:
                    continue
                hs_sb = hsb.tile([65, 512], F32R, tag="hs_sb")
                hd_sb = hsb.tile([65, 512], F32R, tag="hd_sb")
                nc.scalar.activation(
                    hs_sb[0:64, :], hs_ps[:], AF.Relu, bias=b1two_sb[:, 0:1]
                )
                nc.scalar.activation(
                    hd_sb[0:64, :], hd_ps[:], AF.Relu, bias=b1two_sb[:, 1:2]
                )
                nc.sync.dma_start(hs_sb[64:65, :], ones_d[:])
                nc.sync.dma_start(hd_sb[64:65, :], ones_d[:])

            # per-subtile: W + V matmuls
            if oplimit < 4:
                continue
            lhs_d = hd_sb[:, c * 128 : (c + 1) * 128]
            lhs_s = hs_sb[:, c * 128 : (c + 1) * 128]
            wv_halves = []
            if BF16:
                wv_sb = wvs.tile([128, 1024], WDT, tag="wv_sb")
            for half in range(2):
                wh = ps_w.tile([128, 512], F32, tag="wh")
                nc.tensor.matmul(wh[:], lhs_d,
                                 wvW_sb[:, half * 512 : (half + 1) * 512],
                                 start=True, stop=True)
                if BF16:
                    nc.scalar.copy(wv_sb[:, half * 512 : (half + 1) * 512], wh[:])
                wv_halves.append(wh)
            # V_d + V_s summed directly in PSUM by the PE
            v_ps = ps_v.tile([128, 128], F32, tag="vv")
            nc.tensor.matmul(v_ps[:], lhs_d.bitcast(F32),
                             wvW_sb[:, 1024:1152].bitcast(F32), start=True, stop=False)
            nc.tensor.matmul(v_ps[:], lhs_s.bitcast(F32),
                             wvW_sb[:, 1152:1280].bitcast(F32), start=False, stop=True)

            if oplimit < 6:
                continue
            xg = xg4[:, c, 0:IN]  # [128, 32]
            # raw_e = sum_i xg * (V_d+V_s)
            tv = sml.tile([128, 128], F32, tag="tv")
            nc.vector.tensor_tensor(
                tv[:].rearrange("p (h i) -> p h i", i=IN),
                v_ps[:].rearrange("p (h i) -> p h i", i=IN),
                xg.unsqueeze(1).broadcast_to([128, 4, IN]),
                op=OP.mult,
            )
            raw = sml.tile([128, 4], F32, tag="raw")
            nc.vector.tensor_reduce(
                raw[:], tv[:].rearrange("p (h i) -> p h i", i=IN), axis=X, op=OP.add
            )
            # leaky relu fused: lk = max(raw*NEG, raw)
            lk = sml.tile([128, 4], F32, tag="lk")
            nc.vector.scalar_tensor_tensor(lk[:], raw[:], NEG, raw[:],
                                           op0=OP.mult, op1=OP.max)
            if oplimit < 7:
                continue
            tmp3 = t3p.tile([128, 1028], WDT, tag="tmp3")
            nc.scalar.activation(tmp3[:, 1024:1028], lk[:], AF.Exp, bias=esh_sb[:])
            ex = tmp3[:, 1024:1028]
            # xg (x) ex outer product [128, (h, i)]
            xex = sml.tile([128, 4, IN], WDT, tag="xex")
            nc.vector.tensor_tensor(
                xex[:],
                xg.unsqueeze(1).broadcast_to([128, 4, IN]),
                ex.unsqueeze(2).broadcast_to([128, 4, IN]),
                op=OP.mult,
            )
            # tmp3 = W * xex (broadcast over o_l)
            if BF16:
                nc.vector.tensor_tensor(
                    tmp3[:, 0:1024].rearrange("p (h o i) -> p h o i", h=4, o=8),
                    wv_sb[:].rearrange("p (h o i) -> p h o i", h=4, o=8),
                    xex[:].unsqueeze(2).broadcast_to([128, 4, 8, IN]),
                    op=OP.mult,
                )
            else:
                for half in range(2):
                    nc.vector.tensor_tensor(
                        tmp3[:, half * 512 : (half + 1) * 512].rearrange(
                            "p (h o i) -> p h o i", h=2, o=8),
                        wv_halves[half][:].rearrange("p (h o i) -> p h o i", h=2, o=8),
                        xex[:, 2 * half : 2 * half + 2, :].unsqueeze(2).broadcast_to(
                            [128, 2, 8, IN]),
                        op=OP.mult,
                    )
            if oplimit < 8:
                continue
            # one-hot segment matrix
            S = sml.tile([128, 128], WDT, tag="S")
            nc.vector.tensor_tensor(
                S[:],
                iota_sb[:],
                drel_sb[:, sub : sub + 1].broadcast_to([128, 128]),
                op=OP.is_equal,
            )
            if j == 0:
                aggr_ps = ps_ag.tile([128, 1024], F32, tag="aggr")
                den_ps = ps_dn.tile([128, 4], F32, tag="den")
            S_mm = S[:]
            t3_mm = tmp3[:]
            nc.tensor.matmul(
                aggr_ps[:, 0:512], S_mm, t3_mm[:, 0:512], start=(j == 0), stop=(j == G - 1)
            )
            nc.tensor.matmul(
                aggr_ps[:, 512:1024], S_mm, t3_mm[:, 512:1024], start=(j == 0), stop=(j == G - 1)
            )
            nc.tensor.matmul(
                den_ps[:], S_mm, t3_mm[:, 1024:1028], start=(j == 0), stop=(j == G - 1)
            )

            if j == G - 1:
                # finalize window w
                sxj = outp.tile([128, 32], F32, tag="sxj")
                nc.vector.tensor_reduce(
                    sxj[:],
                    aggr_ps[:, 0:1024].rearrange("p (ho i) -> p ho i", i=IN),
                    axis=X,
                    op=OP.add,
                )
                # x_self path
                xT_ps = ps_h.tile([32, 128], F32, tag="hs")
                nc.tensor.transpose(
                    xT_ps[:], xwin_sb[:, w * IN : (w + 1) * IN], ident_sb[:]
                )
                xT_sb = outp.tile([32, 128], F32, tag="xT")
                nc.scalar.copy(xT_sb[:], xT_ps[:])
                xs_ps = ps_h.tile([128, 36], F32, tag="hd")
                nc.tensor.matmul(xs_ps[:], xT_sb[:], selfcat_sb[:], start=True, stop=True)
                # exp(leaky(raw_s))
                rs2 = sml.tile([128, 4], F32, tag="lk2")
                nc.vector.tensor_scalar(rs2[:], xs_ps[:, 32:36], NEG, None, op0=OP.mult)
                rs = sml.tile([128, 4], F32, tag="lk")
                nc.vector.tensor_tensor(rs[:], xs_ps[:, 32:36], rs2[:], op=OP.max)
                exs = sml.tile([128, 4], F32, tag="ex")
                nc.scalar.activation(exs[:], rs[:], AF.Exp, bias=esh_sb[:])
                # numer = sxj + exs * x_self ; den = denom + exs
                t1 = outp.tile([128, 4, 8], F32, tag="t1")
                nc.vector.tensor_tensor(
                    t1[:],
                    xs_ps[:, 0:32].rearrange("p (h o) -> p h o", o=8),
                    exs[:].unsqueeze(2).broadcast_to([128, 4, 8]),
                    op=OP.mult,
                )
                num = outp.tile([128, 32], F32, tag="num")
                nc.vector.tensor_tensor(
                    num[:], sxj[:], t1[:].rearrange("p h o -> p (h o)"), op=OP.add
                )
                den = outp.tile([128, 4], F32, tag="den")
                nc.vector.tensor_tensor(den[:], den_ps[:], exs[:], op=OP.add)
                rec = outp.tile([128, 4], F32, tag="rec")
                nc.vector.reciprocal(rec[:], den[:])
                o1 = outp.tile([128, 32], F32, tag="o1")
                nc.vector.tensor_tensor(
                    o1[:].rearrange("p (h o) -> p h o", o=8),
                    num[:].rearrange("p (h o) -> p h o", o=8),
                    rec[:].unsqueeze(2).broadcast_to([128, 4, 8]),
                    op=OP.mult,
                )
                o2 = outp.tile([128, 32], F32, tag="o2")
                nc.vector.tensor_tensor(o2[:], o1[:], brep_sb[:], op=OP.add)
                nc.sync.dma_start(out_d[w * 128 : (w + 1) * 128, :], o2[:])

    nc.compile()
    return nc


_CACHE = {}


def _get_program(key, meta):
    if key not in _CACHE:
        _CACHE[key] = _build_program(
            meta["G"], meta["nsub"], meta["ntile"], meta["nwin"], meta["npc"],
            meta["nn"],
        )
    return _CACHE[key]


def kernel(**inputs) -> np.ndarray:
    nn = inputs["x"].shape[0]
    npc = (nn + NCORES - 1) // NCORES  # 2500
    nwin = (npc + 127) // 128  # 20
    in_maps, meta = _host_prep(inputs, npc, nwin, NCORES)
    nc = _get_program((meta["G"], meta["nsub"]), meta)
    trace = bool(int(os.environ.get("KERNEL_TRACE", "0")))
    res = run_bass_kernel_spmd(
        nc, in_maps, core_ids=list(range(NCORES)), trace=trace
    )
    global LAST_RESULTS
    LAST_RESULTS = res
    out = np.empty((nn, HO), np.float32)
    for k in range(NCORES):
        take = min(npc, nn - k * npc)
        out[k * npc : k * npc + take] = res.results[k]["out_c"][:take]
    return out


LAST_RESULTS = None


# revision 41
# speedup vs baseline: 1.0285x; 1.0285x over previous
"""DGAT (dynamic-weight GAT) Trainium2 kernel, 8-core edge/node-parallel.

Strategy:
  - Host: sort edges by destination node; shard nodes (and their incoming
    edges) across 8 cores in fixed 2500-node ranges -> fully disjoint, no
    collectives. Within a core, nodes are processed in 128-node windows;
    each window's edges are padded to G*128 slots (G = compile-time max).
  - Device per 128-edge subtile:
      layer1 (PE)  : h_src/h_dst = relu(ef @ w1 + b1)   [64, 512] psum
      W+V (PE)     : [w2d_om | A1d | A1s] matmuls with ones-row bias trick
      bmm (DVE)    : per-edge x_j via broadcast-mult, att-contractions for
                     raw logits via host-precomputed A1 matrices
      softmax      : exp without segment-max (raw range is bounded ~[-3,11]),
                     denominator accumulated alongside messages
      aggregation  : one-hot S matrix (iota==dst_rel) segment-matmul on PE,
                     accumulating G subtiles into a [128 nodes, 1028] PSUM
                     tile; per-window reduce + self-loop terms + divide.
"""
import sys
import os

sys.path.insert(0, "/opt/trn_rl_repo")

import numpy as np
from contextlib import ExitStack

import concourse.bass as bass
import concourse.tile as tile
import concourse.mybir as mybir
from concourse import bacc
from concourse.bass_utils import run_bass_kernel_spmd

F32 = mybir.dt.float32
F32R = mybir.dt.float32r
I16 = mybir.dt.int16
AF = mybir.ActivationFunctionType
OP = mybir.AluOpType
X = mybir.AxisListType.X

N, IN, E = 20000, 32, 200000
EDGE_IN, HID, HEADS, OUT = 16, 64, 4, 8
HO = HEADS * OUT
NEG = 0.2
NCORES = 8


def _host_prep(inp, npc, nwin, ncores):
    """Sort/shard/pad edges; build all per-core device arrays."""
    x = np.ascontiguousarray(inp["x"], dtype=np.float32)
    ef = np.ascontiguousarray(inp["edge_feats"], dtype=np.float32)
    ei = np.asarray(inp["edge_index"]).astype(np.int64)
    dst, src = ei[0], ei[1]
    nn = x.shape[0]
    ne = ef.shape[0]

    order = np.argsort(dst, kind="stable")
    dst_s = dst[order]
    src_s = src[order]
    core_of = dst_s // npc
    win_of = (dst_s % npc) // 128
    gwin = core_of * nwin + win_of
    # index within each (core, window) group (edges are sorted by gwin)
    counts = np.bincount(gwin, minlength=ncores * nwin)
    starts = np.r_[0, np.cumsum(counts)][:-1]
    idx_in_win = np.arange(ne) - starts[gwin]
    G = int(np.ceil(counts.max() / 128))
    nsub = nwin * G
    ntile = (nsub + 3) // 4
    nsub = ntile * 4  # pad subtiles to gather-tile granularity
    slots = ntile * 512

    slot_src = np.zeros((ncores, slots), np.int16)
    slot_rel = np.full((ncores, slots), -1.0, np.float32)
    slot_ef = np.zeros((ncores, slots, EDGE_IN), np.float32)
    s_global = win_of * (G * 128) + idx_in_win
    slot_src[core_of, s_global] = src_s.astype(np.int16)
    slot_rel[core_of, s_global] = ((dst_s % npc) % 128).astype(np.float32)
    slot_ef[core_of, s_global] = ef[order]

    # transposed layout: feature on partitions 0-15, slot on free dim
    ef_p = np.ascontiguousarray(slot_ef.transpose(0, 2, 1))  # [ncores, 16, slots]
    gidx16 = slot_src.reshape(ncores, slots // 16, 16).transpose(0, 2, 1)
    gidx = np.ascontiguousarray(
        np.tile(gidx16, (1, 8, 1))
    )  # [ncores, 128, slots/16] — idx pattern replicated per Q7 core
    drel = np.ascontiguousarray(
        slot_rel.reshape(ncores, nsub, 128).transpose(0, 2, 1)
    )  # [ncores, 128, nsub]

    # x padded to 256B rows for dma_gather
    xpad = np.zeros((nn, 64), np.float32)
    xpad[:, :IN] = x

    # per-core x windows, pre-arranged [p, (w, f)]
    node_slots = nwin * 128
    xwin = np.zeros((ncores, 128, nwin * IN), np.float32)
    for k in range(ncores):
        lo = k * npc
        take = min(npc, nn - lo)
        xw = np.zeros((node_slots, IN), np.float32)
        xw[:take] = x[lo : lo + take]
        xwin[k] = xw.reshape(nwin, 128, IN).transpose(1, 0, 2).reshape(128, nwin * IN)

    f32 = lambda k: np.asarray(inp[k], dtype=np.float32)
    w1s, w1d = f32("src_w1"), f32("dst_w1")
    b1s, b1d = f32("src_b1"), f32("dst_b1")
    w2s, w2d = f32("src_w2"), f32("dst_w2")
    b2s, b2d = f32("src_b2"), f32("dst_b2")
    selfw = f32("self_weights")
    att = f32("att")[0]  # [4, 16]
    bias = f32("bias")
    att1, att2 = att[:, :OUT], att[:, OUT:]

    w1cat = np.concatenate([w1s, w1d], axis=1).astype(np.float32)  # [16, 128]
    b1two = np.stack([b1s, b1d], axis=1).astype(np.float32)  # [64, 2]

    # o-major w2d + bias row
    w2d_om = w2d.reshape(HID, IN, HO).transpose(0, 2, 1).reshape(HID, HO * IN)
    b2d_om = b2d.reshape(IN, HO).T.reshape(1, HO * IN)
    # A1 matrices (h-major, i-inner) + bias rows
    w2s4 = w2s.reshape(HID, IN, HEADS, OUT)
    w2d4 = w2d.reshape(HID, IN, HEADS, OUT)
    A1d = np.einsum("zihg,hg->zhi", w2d4, att2).reshape(HID, HEADS * IN)
    A1s = np.einsum("zihg,hg->zhi", w2s4, att1).reshape(HID, HEADS * IN)
    A1d_b = np.einsum("ihg,hg->hi", b2d.reshape(IN, HEADS, OUT), att2).reshape(1, -1)
    A1s_b = np.einsum("ihg,hg->hi", b2s.reshape(IN, HEADS, OUT), att1).reshape(1, -1)
    wvW = np.concatenate(
        [
            np.concatenate([w2d_om, b2d_om], 0),
            np.concatenate([A1d, A1d_b], 0),
            np.concatenate([A1s, A1s_b], 0),
        ],
        axis=1,
    ).astype(np.float32)  # [65, 1280]

    rawsW = np.einsum(
        "ihg,hg->ih", selfw.reshape(IN, HEADS, OUT), att1 + att2
    ).astype(np.float32)
    selfcat = np.concatenate([selfw, rawsW], axis=1).astype(np.float32)  # [32, 36]

    ident = np.eye(128, dtype=np.float32)
    iota_r = np.tile(np.arange(128, dtype=np.float32)[None, :], (128, 1))
    brep = np.tile(bias[None, :], (128, 1)).astype(np.float32)

    in_maps = []
    for k in range(ncores):
        in_maps.append(
            {
                "ef_p": np.ascontiguousarray(ef_p[k]),
                "xpad": xpad,
                "gidx": np.ascontiguousarray(gidx[k]),
                "drel": np.ascontiguousarray(drel[k]),
                "xwin": np.ascontiguousarray(xwin[k]),
                "w1cat": w1cat,
                "b1two": b1two,
                "wvW": wvW,
                "selfcat": selfcat,
                "ident": ident,
                "iota_r": iota_r,
                "brep": brep,
                "ones_r": np.ones((1, 512), np.float32),
            }
        )
    meta = dict(G=G, nsub=nsub, ntile=ntile, nwin=nwin, npc=npc, nn=nn)
    return in_maps, meta


def _build_program(G, nsub, ntile, nwin, npc, nn):
    BF16 = bool(int(os.environ.get("KERNEL_BF16", "1")))
    BF = mybir.dt.float16
    WDT = BF if BF16 else F32
    ESHIFT = -5.0 if BF16 else 0.0
    nc = bacc.Bacc("TRN2", target_bir_lowering=False, debug=False)
    node_slots = nwin * 128

    ef_d = nc.dram_tensor("ef_p", [16, ntile * 512], F32R, kind="ExternalInput")
    xpad_d = nc.dram_tensor("xpad", [nn, 64], F32, kind="ExternalInput")
    gidx_d = nc.dram_tensor("gidx", [128, ntile * 32], I16, kind="ExternalInput")
    drel_d = nc.dram_tensor("drel", [128, nsub], F32, kind="ExternalInput")
    xwin_d = nc.dram_tensor("xwin", [128, nwin * IN], F32, kind="ExternalInput")
    w1cat_d = nc.dram_tensor("w1cat", [16, 128], F32R, kind="ExternalInput")
    b1two_d = nc.dram_tensor("b1two", [64, 2], F32, kind="ExternalInput")
    wvW_d = nc.dram_tensor("wvW", [65, 1280], F32R, kind="ExternalInput")
    selfcat_d = nc.dram_tensor("selfcat", [32, 36], F32, kind="ExternalInput")
    ident_d = nc.dram_tensor("ident", [128, 128], F32, kind="ExternalInput")
    iota_d = nc.dram_tensor("iota_r", [128, 128], F32, kind="ExternalInput")
    brep_d = nc.dram_tensor("brep", [128, 32], F32, kind="ExternalInput")
    ones_d = nc.dram_tensor("ones_r", [1, 512], F32R, kind="ExternalInput")
    out_d = nc.dram_tensor("out_c", [node_slots, IN], F32, kind="ExternalOutput")

    with tile.TileContext(nc) as tc, ExitStack() as ctx:
        const = ctx.enter_context(tc.tile_pool(name="const", bufs=1))
        xgp = ctx.enter_context(tc.tile_pool(name="xgp", bufs=3))
        hsb = ctx.enter_context(tc.tile_pool(name="hsb", bufs=3))
        wvs = ctx.enter_context(tc.tile_pool(name="wvs", bufs=3))
        sml = ctx.enter_context(tc.tile_pool(name="sml", bufs=4))
        t3p = ctx.enter_context(tc.tile_pool(name="t3p", bufs=3))
        outp = ctx.enter_context(tc.tile_pool(name="outp", bufs=2))
        ps_h = ctx.enter_context(tc.tile_pool(name="ps_h", bufs=1, space="PSUM"))
        ps_w = ctx.enter_context(tc.tile_pool(name="ps_w", bufs=2, space="PSUM"))
        ps_v = ctx.enter_context(tc.tile_pool(name="ps_v", bufs=1, space="PSUM"))
        ps_ag = ctx.enter_context(tc.tile_pool(name="ps_ag", bufs=1, space="PSUM"))
        ps_dn = ctx.enter_context(tc.tile_pool(name="ps_dn", bufs=1, space="PSUM"))

        def load_const(name, dram, shape, dtype=F32):
            t = const.tile(shape, dtype, tag=name)
            nc.sync.dma_start(t[:], dram[:])
            return t

        ef_sb = load_const("ef_sb", ef_d, [16, ntile * 512], F32R)
        gidx_sb = load_const("gidx_sb", gidx_d, [128, ntile * 32], I16)
        drel_sb = load_const("drel_sb", drel_d, [128, nsub])
        xwin_sb = load_const("xwin_sb", xwin_d, [128, nwin * IN])
        w1cat_sb = load_const("w1cat_sb", w1cat_d, [16, 128], F32R)
        b1two_sb = load_const("b1two_sb", b1two_d, [64, 2])
        wvW_sb = load_const("wvW_sb", wvW_d, [65, 1280], F32R)
        selfcat_sb = load_const("selfcat_sb", selfcat_d, [32, 36])
        ident_sb = load_const("ident_sb", ident_d, [128, 128])
        iota_sb = load_const("iota_sb", iota_d, [128, 128])
        brep_sb = load_const("brep_sb", brep_d, [128, 32])
        esh_sb = const.tile([128, 1], F32, tag="esh")
        nc.gpsimd.memset(esh_sb[:], ESHIFT)

        xg4 = None
        hs_sb = hd_sb = None
        aggr_ps = None

        sublimit = int(os.environ.get("KERNEL_SUBLIMIT", str(nsub)))
        oplimit = int(os.environ.get("KERNEL_OPLIMIT", "99"))
        for sub in range(min(nsub, sublimit)):
            t, c = sub // 4, sub % 4
            w, j = sub // G, sub % G
            if w >= nwin:
                break
            if c == 0:
                # gather 512 edges' x rows
                xg4 = xgp.tile([128, 4, 64], F32, tag="xg4")
                nc.gpsimd.dma_gather(
                    xg4[:], xpad_d[:], gidx_sb[:, t * 32 : (t + 1) * 32], 512, 512, 64
                )
                if oplimit < 2:
                    continue
                # layer 1: one N=512 f32r matmul per net -> h [64, 512] psum
                hs_ps = ps_h.tile([64, 512], F32, tag="hs")
                hd_ps = ps_h.tile([64, 512], F32, tag="hd")
                rhs = ef_sb[:, t * 512 : (t + 1) * 512]
                for net, ps in ((0, hs_ps), (1, hd_ps)):
                    nc.tensor.matmul(
                        ps[:],
                        w1cat_sb[:, net * 64 : (net + 1) * 64],
                        rhs,
                        start=True,
                        stop=True,
                    )
                if oplimit < 3:
                    continue
                hs_sb = hsb.tile([65, 512], F32R, tag="hs_sb")
                hd_sb = hsb.tile([65, 512], F32R, tag="hd_sb")
                nc.scalar.activation(
                    hs_sb[0:64, :], hs_ps[:], AF.Relu, bias=b1two_sb[:, 0:1]
                )
                nc.scalar.activation(
                    hd_sb[0:64, :], hd_ps[:], AF.Relu, bias=b1two_sb[:, 1:2]
                )
                nc.sync.dma_start(hs_sb[64:65, :], ones_d[:])
                nc.sync.dma_start(hd_sb[64:65, :], ones_d[:])

            # per-subtile: W + V matmuls
            if oplimit < 4:
                continue
            lhs_d = hd_sb[:, c * 128 : (c + 1) * 128]
            lhs_s = hs_sb[:, c * 128 : (c + 1) * 128]
            wv_halves = []
            if BF16:
                wv_sb = wvs.tile([128, 1024], WDT, tag="wv_sb")
            for half in range(2):
                wh = ps_w.tile([128, 512], F32, tag="wh")
                nc.tensor.matmul(wh[:], lhs_d,
                                 wvW_sb[:, half * 512 : (half + 1) * 512],
                                 start=True, stop=True)
                if BF16:
                    nc.scalar.copy(wv_sb[:, half * 512 : (half + 1) * 512], wh[:])
                wv_halves.append(wh[:])
            # V_d + V_s summed directly in PSUM by the PE
            v_ps = ps_v.tile([128, 128], F32, tag="vv")
            nc.tensor.matmul(v_ps[:], lhs_d.bitcast(F32),
                             wvW_sb[:, 1024:1152].bitcast(F32), start=True, stop=False)
            nc.tensor.matmul(v_ps[:], lhs_s.bitcast(F32),
                             wvW_sb[:, 1152:1280].bitcast(F32), start=False, stop=True)

            if oplimit < 6:
                continue
            xg = xg4[:, c, 0:IN]  # [128, 32]
            # raw_e = sum_i xg * (V_d+V_s)
            tv = sml.tile([128, 128], F32, tag="tv")
            nc.vector.tensor_tensor(
                tv[:].rearrange("p (h i) -> p h i", i=IN),
                v_ps[:].rearrange("p (h i) -> p h i", i=IN),
                xg.unsqueeze(1).broadcast_to([128, 4, IN]),
                op=OP.mult,
            )
            raw = sml.tile([128, 4], F32, tag="raw")
            nc.vector.tensor_reduce(
                raw[:], tv[:].rearrange("p (h i) -> p h i", i=IN), axis=X, op=OP.add
            )
            # leaky relu fused: lk = max(raw*NEG, raw)
            lk = sml.tile([128, 4], F32, tag="lk")
            nc.vector.scalar_tensor_tensor(lk[:], raw[:], NEG, raw[:],
                                           op0=OP.mult, op1=OP.max)
            if oplimit < 7:
                continue
            tmp3 = t3p.tile([128, 1028], WDT, tag="tmp3")
            nc.scalar.activation(tmp3[:, 1024:1028], lk[:], AF.Exp, bias=esh_sb[:])
            ex = tmp3[:, 1024:1028]
            # xg (x) ex outer product [128, (h, i)]
            xex = sml.tile([128, 4, IN], WDT, tag="xex")
            nc.vector.tensor_tensor(
                xex[:],
                xg.unsqueeze(1).broadcast_to([128, 4, IN]),
                ex.unsqueeze(2).broadcast_to([128, 4, IN]),
                op=OP.mult,
            )
            # tmp3 = W * xex (broadcast over o_l)
            if BF16:
                nc.vector.tensor_tensor(
                    tmp3[:, 0:1024].rearrange("p (h o i) -> p h o i", h=4, o=8),
                    wv_sb[:].rearrange("p (h o i) -> p h o i", h=4, o=8),
                    xex[:].unsqueeze(2).broadcast_to([128, 4, 8, IN]),
                    op=OP.mult,
                )
            else:
                for half in range(2):
                    nc.vector.tensor_tensor(
                        tmp3[:, half * 512 : (half + 1) * 512].rearrange(
                            "p (h o i) -> p h o i", h=2, o=8),
                        wv_halves[half].rearrange("p (h o i) -> p h o i", h=2, o=8),
                        xex[:, 2 * half : 2 * half + 2, :].unsqueeze(2).broadcast_to(
                            [128, 2, 8, IN]),
                        op=OP.mult,
                    )
            if oplimit < 8:
                continue
            # one-hot segment matrix
            S = sml.tile([128, 128], WDT, tag="S")
            nc.vector.tensor_tensor(
                S[:],
                iota_sb[:],
                drel_sb[:, sub : sub + 1].broadcast_to([128, 128]),
                op=OP.is_equal,
            )
            if j == 0:
                aggr_ps = ps_ag.tile([128, 1024], F32, tag="aggr")
                den_ps = ps_dn.tile([128, 4], F32, tag="den")
            S_mm = S[:]
            t3_mm = tmp3[:]
            nc.tensor.matmul(
                aggr_ps[:, 0:512], S_mm, t3_mm[:, 0:512], start=(j == 0), stop=(j == G - 1)
            )
            nc.tensor.matmul(
                aggr_ps[:, 512:1024], S_mm, t3_mm[:, 512:1024], start=(j == 0), stop=(j == G - 1)
            )
            nc.tensor.matmul(
                den_ps[:], S_mm, t3_mm[:, 1024:1028], start=(j == 0), stop=(j == G - 1)
            )

            if j == G - 1:
                # finalize window w
                sxj = outp.tile([128, 32], F32, tag="sxj")
                nc.vector.tensor_reduce(
                    sxj[:],
                    aggr_ps[:, 0:1024].rearrange("p (ho i) -> p ho i", i=IN),
                    axis=X,
                    op=OP.add,
                )
                # x_self path
                xT_ps = ps_h.tile([32, 128], F32, tag="hs")
                nc.tensor.transpose(
                    xT_ps[:], xwin_sb[:, w * IN : (w + 1) * IN], ident_sb[:]
                )
                xT_sb = outp.tile([32, 128], F32, tag="xT")
                nc.scalar.copy(xT_sb[:], xT_ps[:])
                xs_ps = ps_h.tile([128, 36], F32, tag="hd")
                nc.tensor.matmul(xs_ps[:], xT_sb[:], selfcat_sb[:], start=True, stop=True)
                # exp(leaky(raw_s))
                rs0 = sml.tile([128, 4], F32, tag="lk2")
                nc.vector.tensor_copy(rs0[:], xs_ps[:, 32:36])
                rs = sml.tile([128, 4], F32, tag="lk")
                nc.vector.scalar_tensor_tensor(rs[:], rs0[:], NEG, rs0[:],
                                               op0=OP.mult, op1=OP.max)
                exs = sml.tile([128, 4], F32, tag="ex")
                nc.scalar.activation(exs[:], rs[:], AF.Exp, bias=esh_sb[:])
                # numer = sxj + exs * x_self ; den = denom + exs
                t1 = outp.tile([128, 4, 8], F32, tag="t1")
                nc.vector.tensor_tensor(
                    t1[:],
                    xs_ps[:, 0:32].rearrange("p (h o) -> p h o", o=8),
                    exs[:].unsqueeze(2).broadcast_to([128, 4, 8]),
                    op=OP.mult,
                )
                num = outp.tile([128, 32], F32, tag="num")
                nc.vector.tensor_tensor(
                    num[:], sxj[:], t1[:].rearrange("p h o -> p (h o)"), op=OP.add
                )
                den = outp.tile([128, 4], F32, tag="den")
                nc.vector.tensor_tensor(den[:], den_ps[:], exs[:], op=OP.add)
                rec = outp.tile([128, 4], F32, tag="rec")
                nc.vector.reciprocal(rec[:], den[:])
                o1 = outp.tile([128, 32], F32, tag="o1")
                nc.vector.tensor_tensor(
                    o1[:].rearrange("p (h o) -> p h o", o=8),
                    num[:].rearrange("p (h o) -> p h o", o=8),
                    rec[:].unsqueeze(2).broadcast_to([128, 4, 8]),
                    op=OP.mult,
                )
                o2 = outp.tile([128, 32], F32, tag="o2")
                nc.vector.tensor_tensor(o2[:], o1[:], brep_sb[:], op=OP.add)
                nc.sync.dma_start(out_d[w * 128 : (w + 1) * 128, :], o2[:])

    nc.compile()
    return nc


_CACHE = {}


def _get_program(key, meta):
    if key not in _CACHE:
        _CACHE[key] = _build_program(
            meta["G"], meta["nsub"], meta["ntile"], meta["nwin"], meta["npc"],
            meta["nn"],
        )
    return _CACHE[key]


def kernel(**inputs) -> np.ndarray:
    nn = inputs["x"].shape[0]
    npc = (nn + NCORES - 1) // NCORES  # 2500
    nwin = (npc + 127) // 128  # 20
    in_maps, meta = _host_prep(inputs, npc, nwin, NCORES)
    nc = _get_program((meta["G"], meta["nsub"]), meta)
    trace = bool(int(os.environ.get("KERNEL_TRACE", "0")))
    res = run_bass_kernel_spmd(
        nc, in_maps, core_ids=list(range(NCORES)), trace=trace
    )
    global LAST_RESULTS
    LAST_RESULTS = res
    out = np.empty((nn, HO), np.float32)
    for k in range(NCORES):
        take = min(npc, nn - k * npc)
        out[k * npc : k * npc + take] = res.results[k]["out_c"][:take]
    return out


LAST_RESULTS = None
